# revision 96
# baseline (speedup 1.0000x reference)
"""Trainium2 Bass kernel for nn_CTAttention2 (DPC-KNN cluster attention).

Self-contained: accepts FULL inputs (B=8,N=1024,C=256), shards batch across
8 NeuronCores (one batch element per core), runs a fully fused Bass/Tile
kernel per core, and gathers the full output.

v2 redesign vs the fp32 baseline (272us): every large matmul now runs in
f32r (1 cycle/row vs fp32's 4) and the same-cluster attention mask is
restructured so it stays EXACT in any PE precision:

  - cluster ids c0 in 0..255 are split into digits a=c0/16, b=c0%16 and
    encoded as 32 one-hot rows (products only 0/1/128/-256, exactly
    representable).  kTm/qTm are [97,N]: rows 0..31 one-hot (k side x128,
    q side x1), row 32 the -256 constant, rows 33..96 the head dims.  A
    same-cluster pair accumulates +128+128-256 = 0 exactly; a distinct
    pair is <= -128 so exp() flushes to ~0 (vs reference's exp(-1e9)).
  - d2 matmuls run on f32r with a hi/lo split of sq (granularity 2^-2 via
    the +-2^21 trick) across four augmentation rows, so the sq terms stay
    exact and only the x.x products see f32r rounding (~1e-4 abs on d2).
  - digit rows are recovered without mod/floor (unsupported ALU ops):
    pack = 48*a + c0 = 64a+b is gathered per token by the assignment
    scatter, then a = sum of 15 range compares (PE partition-sum), and
    b = pack - 64a.
  - density/score2 compare broadcasts stay exact fp32 (a rounded broadcast
    would corrupt self-compares in the is_gt masks); {0,1}/integer/recip
    broadcasts use f32r ones (1 cyc/row).
  - reduces and plain tensor_tensor ops move to the Pool engine; PSUM
    evacuations are split between Act and DVE to keep the score-exp loop
    (Act-bound) and the clustering chain (DVE-bound) short.
"""

import os
import sys

for _p in ("/opt/trn_rl_repo", "/root/.axon_site/_ro/trn_rl_repo"):
    if os.path.isdir(_p) and _p not in sys.path:
        sys.path.insert(0, _p)

import numpy as np

import concourse.bass as bass
import concourse.tile as tile
from concourse import mybir
from concourse.bass_types import SemaphoreHandle
from concourse.vector_clock import ScopedClock

B, N, C = 8, 1024, 256
H, D = 4, 64
NBLK = N // 128
K5 = 5
EPS = 1e-6
C0 = 8.0          # z = C0 - d2/256
BIGM = 128.0      # one-hot mask weight; exp(s - 128) == 0.0 for |s|<=~20
SPLIT = float(2 ** 21)  # hi/lo split constant for sq (granularity 0.25)
FP = mybir.dt.float32
FR = mybir.dt.float32r
BF = mybir.dt.bfloat16
A = mybir.AluOpType
AF = mybir.ActivationFunctionType
AX = mybir.AxisListType


# ---------------------------------------------------------------------------
# Workaround: this walrus build rejects the multi-wait tail Drain emitted by
# TileContext ("Too many sync wait commands").  Emit one single-wait SP
# instruction per outstanding semaphore instead, then a wait-free drain.
# ---------------------------------------------------------------------------
def _patched_drain_and_barrier(self, tick_clock, wait_clock):
    nc = self.nc
    probe = mybir.InstNoOp(name=f"drain-probe-{nc.next_id()}", ins=[], outs=[])
    probe.engine = mybir.EngineType.SP
    wait_clock.add_sem_waits(probe, ScopedClock({None: tick_clock.global_clock}))
    if probe.sync_info is not None:
        for w in probe.sync_info.on_wait:
            assert w.wait_mode == "sem-ge-imm", w
            nc.sync.wait_ge(SemaphoreHandle(w.ant_name, w.id), w.wait_value)
    nc.sync.drain()
    nc.all_engine_barrier()
    popped = nc._tile_sem_poison_stack.pop()
    assert popped is self._sem_poison
    nc.clear_and_free_semaphores(list(self.sems.allocated().values()))
    nc.all_engine_barrier()


def _install_drain_patch():
    tile.TileContext._drain_and_barrier = _patched_drain_and_barrier


# ---------------------------------------------------------------------------
# Workaround #2: the same walrus build caps the number of sync-wait commands
# per instruction.  Post-process the BIR JSON just before the walrus call:
# move excess waits onto single-wait NoOps inserted immediately before the
# instruction on the same engine.
# ---------------------------------------------------------------------------
_WAIT_CAPS = {"default": 1}


def _split_excess_waits(bir_json):
    import json as _json

    d = _json.loads(bir_json)
    changed = False
    for fn in d.get("functions", []):
        for bb in fn.get("blocks", []):
            out = []
            for inst in bb.get("instructions", []):
                si = inst.get("sync_info")
                waits = (si or {}).get("on_wait") or []
                cap = _WAIT_CAPS.get(inst.get("opcode"), _WAIT_CAPS["default"])
                if len(waits) > cap:
                    keep = waits[-cap:] if cap > 0 else []
                    extra = waits[: len(waits) - cap]
                    for k, w in enumerate(extra):
                        carrier = {
                            "name": f"{inst['name']}__w{k}",
                            "opcode": "EventSemaphore",
                            "engine": inst["engine"],
                            "ins": [],
                            "outs": [],
                            "sync_info": {"on_wait": [w], "on_update": []},
                        }
                        if "debug" in inst:
                            carrier["debug"] = inst["debug"]
                        out.append(carrier)
                    si["on_wait"] = keep
                    changed = True
                out.append(inst)
            bb["instructions"] = out
    if not changed:
        return bir_json
    return _json.dumps(d).encode()


_ORIG_COMPILE = {}


def _install_wait_split_patch():
    import concourse.bass2jax as bass2jax
    import concourse.bass_utils as bass_utils

    if "impl" in _ORIG_COMPILE:
        return
    orig = bass_utils.compile_bir_kernel
    _ORIG_COMPILE["impl"] = orig

    def patched(bir_json, tmpdir, neff_name="file.neff"):
        return orig(_split_excess_waits(bir_json), tmpdir, neff_name=neff_name)

    bass_utils.compile_bir_kernel = patched
    bass2jax.compile_bir_kernel = patched


def _noise_cols():
    """Reference tie-break noise, per core, in [part, blk] layout, x1e-6."""
    import jax
    import jax.numpy as jnp

    with jax.default_device(jax.devices("cpu")[0]):
        u = jax.random.uniform(jax.random.key(42), (B, N), dtype=jnp.float32)
    u = np.asarray(u).astype(np.float32) * np.float32(1e-6)
    # token i = 128*blk + part  ->  [core][part, blk]
    return [np.ascontiguousarray(u[b].reshape(NBLK, 128).T) for b in range(B)]


def build_nc():
    _install_drain_patch()
    _install_wait_split_patch()
    nc = bass.Bass(num_swdge_queues=4)

    x_ext = nc.declare_dram_parameter("x", [N, C], FP, isOutput=False)
    wq_ext = nc.declare_dram_parameter("Wq", [C, C], FP, isOutput=False)
    wk_ext = nc.declare_dram_parameter("Wk", [C, C], FP, isOutput=False)
    wv_ext = nc.declare_dram_parameter("Wv", [C, C], FP, isOutput=False)
    wp_ext = nc.declare_dram_parameter("Wp", [C, C], FP, isOutput=False)
    bp_ext = nc.declare_dram_parameter("bp", [C], FP, isOutput=False)
    noise_ext = nc.declare_dram_parameter("noise", [128, NBLK], FP, isOutput=False)
    ident_ext = nc.declare_dram_parameter("ident", [128, 128], FP, isOutput=False)
    iota64_ext = nc.declare_dram_parameter("iota64", [64, 1], FP, isOutput=False)
    selblk_ext = nc.declare_dram_parameter("selblk", [16, 512], FP, isOutput=False)
    out_ext = nc.declare_dram_parameter("out", [N, C], FP, isOutput=True)

    wexts = {"q": wq_ext, "k": wk_ext, "v": wv_ext, "p": wp_ext}

    with tile.TileContext(nc) as tc:
        with (
            tc.tile_pool(name="consts", bufs=1) as consts,
            tc.tile_pool(name="big", bufs=1) as big,
            tc.tile_pool(name="mid", bufs=1) as mid,
            tc.tile_pool(name="scr", bufs=2) as scr,
            tc.tile_pool(name="psA", bufs=2, space="PSUM") as psA,
            tc.tile_pool(name="psS", bufs=2, space="PSUM") as psS,
            tc.tile_pool(name="psPV", bufs=1, space="PSUM") as psPV,
        ):
            # ---------------- loads ----------------
            xr = big.tile([128, NBLK, C], FP, tag="zbig")  # reused: xr -> z -> PT
            for bb2 in range(4):
                nc.sync.dma_start(
                    out=xr[:, 2 * bb2 : 2 * bb2 + 2, :],
                    in_=x_ext.rearrange("(b p) c -> p b c", p=128)[
                        :, 2 * bb2 : 2 * bb2 + 2, :
                    ],
                )
            ident = consts.tile([128, 128], FP, tag="ident")
            nc.sync.dma_start(out=ident[:], in_=ident_ext[:])
            noise_sb = consts.tile([128, NBLK], FP, tag="noise")
            nc.sync.dma_start(out=noise_sb[:], in_=noise_ext[:])
            bp_row = consts.tile([1, C], FP, tag="bp_row")
            nc.sync.dma_start(out=bp_row[:], in_=bp_ext.rearrange("(a c) -> a c", a=1))
            bp_fr = consts.tile([1, C], FR, tag="bp_fr")
            nc.scalar.copy(out=bp_fr[:], in_=bp_row[:])
            iota64 = consts.tile([64, 1], FP, tag="iota64")
            nc.sync.dma_start(out=iota64[:], in_=iota64_ext[:])
            selblkf = consts.tile([16, 512], FP, tag="selblkf")
            nc.sync.dma_start(out=selblkf[:], in_=selblk_ext[:])
            selblk = consts.tile([16, 512], FR, tag="selblk")
            nc.scalar.copy(out=selblk[:], in_=selblkf[:])

            wraw = {}
            for nm in ("q", "k", "v", "p"):
                t = consts.tile([128, 2, C], FP, tag=f"wraw{nm}", name=f"wraw{nm}")
                nc.sync.dma_start(
                    out=t[:], in_=wexts[nm].rearrange("(t p) c -> p t c", p=128)
                )
                wraw[nm] = t

            # ---------------- [1,N] row storage ----------------
            # all rows live at base partition 0 (engine ops require matching
            # start partitions); p0_* slots are reused across lifetimes.
            # Rows consumed by f32r matmuls are allocated as FR so their
            # producing instruction carries the f32r rounding flag.
            _P0TAG = {
                "brow": ("p0_1", FR),
                "arow": ("p0_3", FR),
                "den": ("p0_3", FP),
                "cm": ("p0_6", FP), "v": ("p0_6", FR),
            }
            p0rows = {}

            def mkrow(name):
                tag, dt = _P0TAG[name]
                p0rows[name] = mid.tile([1, N], dt, tag=tag, name=name)
                ap = p0rows[name][0:1, :]
                return ap if dt is FP else ap.bitcast(FP)

            def row(name):
                ap = p0rows[name][0:1, :]
                return ap if p0rows[name].dtype is FP else ap.bitcast(FP)

            def rowfr(name):
                return p0rows[name][0:1, :]

            ones1 = consts.tile([1, 128], FP, tag="ones1")
            nc.vector.memset(ones1[:], 1.0)
            negbig = consts.tile([128, 1], FP, tag="negbig")
            nc.vector.memset(negbig[:], -2.0 * BIGM)
            ones1r = consts.tile([1, 128], FR, tag="ones1r")
            nc.vector.memset(ones1r[:].bitcast(FP), 1.0)

            def col2row(dst, col_ap, nparts=NBLK, dt=FP):
                # [128, nparts] column tile -> [1, N] row: PE transpose,
                # evacuate (rounding to FR when the row feeds f32r matmuls),
                # then two queue-parallel gather DMAs.
                ptr = psS.tile([128, C], FP, tag="ps_small", name="ptr")
                nc.tensor.transpose(
                    out=ptr[0:nparts, 0:128], in_=col_ap, identity=ident[:]
                )
                t8 = scr.tile([32, 128], dt, tag="scrT", name="t8")
                nc.scalar.copy(out=t8[0:nparts, :], in_=ptr[0:nparts, 0:128])
                half = NBLK // 2
                nc.sync.dma_start(out=dst[:, 0 : N // 2], in_=t8[0:half, :])
                nc.sync.dma_start(out=dst[:, N // 2 : N], in_=t8[half:NBLK, :])

            def replicate(dst, src_row, parts=128, pool=None, evac="act"):
                # PE broadcast: f32r ones[1,parts] x row[1,N] -> psum -> SBUF.
                # f32r is exact for the {0,1}/small-integer/recip rows it is
                # used on here.
                nfree = dst.shape[-1]
                nsl = (nfree + 511) // 512
                pl = pool or psA
                pb = pl.tile(
                    [128, N], FP,
                    tag="ps_big" if pl is psA else "ps_pv", name="pb",
                )
                for n_ in range(nsl):
                    sl = slice(512 * n_, min(512 * (n_ + 1), nfree))
                    nc.tensor.matmul(
                        pb[0:parts, sl], ones1r[:, 0:parts],
                        src_row[:, sl], start=True, stop=True,
                    )
                if evac == "act":
                    nc.scalar.copy(out=dst[:], in_=pb[0:parts, 0:nfree])
                else:
                    nc.vector.tensor_scalar_add(
                        out=dst[:], in0=pb[0:parts, 0:nfree], scalar1=0.0
                    )

            def blockbcast(dst, col_ap_fn):
                # dst[p, 128*jb+jc] = col[jc, jb], exact fp32: per block an
                # Act bias-broadcast ([128,1] col -> [128,128]) + PE transpose.
                for jb in range(NBLK):
                    stage = scr.tile([128, 128], FP, tag="scrT", name="bbst")
                    nc.scalar.activation(
                        out=stage[:], in_=ident[:], func=AF.Identity,
                        bias=col_ap_fn(jb), scale=0.0,
                    )
                    pt = psS.tile([128, 128], FP, tag="ps_small")
                    nc.tensor.transpose(out=pt[:], in_=stage[:], identity=ident[:])
                    nc.scalar.copy(
                        out=dst[:, 128 * jb : 128 * (jb + 1)], in_=pt[:]
                    )

            # ---------------- sq = rowsum(x^2) (DVE, early) ----------------
            sq_col = consts.tile([128, NBLK], FP, tag="sq_col")
            for b in range(NBLK):
                scx = scr.tile([128, C], FP, tag="scrB")
                nc.vector.scalar_tensor_tensor(
                    out=scx[:],
                    in0=xr[:, b, :],
                    scalar=1.0,
                    in1=xr[:, b, :],
                    op0=A.mult,
                    op1=A.mult,
                    accum_out=sq_col[:, b : b + 1],
                )
            # hi/lo split of sq: hi = (sq + 2^21) - 2^21 (multiple of 0.25);
            # stored directly as -hi/2, -lo/2 column stacks for the j-side aug
            hilo = mid.tile([128, 16], FP, tag="hilo")
            nc.vector.tensor_scalar(
                out=hilo[:, 0:NBLK], in0=sq_col[:], scalar1=SPLIT, scalar2=-SPLIT,
                op0=A.add, op1=A.add,
            )
            nc.vector.tensor_tensor(
                out=hilo[:, NBLK:16], in0=sq_col[:], in1=hilo[:, 0:NBLK],
                op=A.subtract,
            )
            nc.vector.tensor_scalar_mul(out=hilo[:], in0=hilo[:], scalar1=-0.5)
            # z-evac bias: C0 - sq_i/256 per partition (exact fp32)
            zbias_col = mid.tile([128, NBLK], FP, tag="zbias_col")
            nc.vector.tensor_scalar(
                out=zbias_col[:], in0=sq_col[:], scalar1=-1.0 / 256.0, scalar2=C0,
                op0=A.mult, op1=A.add,
            )

            # ---------------- transposes (PE) ----------------
            xT = consts.tile([128, 2, N], FP, tag="xT")
            for hf in range(2):
                for t_ in range(2):
                    for r in range(4 * hf, 4 * hf + 4):
                        pt = psS.tile([128, 128], FP, tag="ps_small")
                        nc.tensor.transpose(
                            out=pt[:],
                            in_=xr[:, r, 128 * t_ : 128 * (t_ + 1)],
                            identity=ident[:],
                        )
                        nc.scalar.copy(
                            out=xT[:, t_, 128 * r : 128 * (r + 1)], in_=pt[:]
                        )

            # Veltkamp split at 12 bits: xh passes through f32r exactly,
            # xl carries the residual; x.x via 3 f32r matmuls is fp32-exact.
            # Chunked by column halves (half-major, all on DVE) so d2's first
            # 512-slice can start once cols 0:512 of both k-halves are split.
            xh_t = consts.tile([128, 2, N], FR, tag="xh")
            xl_t = consts.tile([128, 2, N], FR, tag="xl")
            for hf in range(2):
                hsl = slice(512 * hf, 512 * (hf + 1))
                for k in range(2):
                    # last chunk on Pool (idle in the prologue): DVE and Pool
                    # finish ~together so d2 streams without the hf=1 stall
                    eng = nc.vector if not (hf == 1 and k == 1) else nc.gpsimd
                    tv = scr.tile([128, 512], FP, tag="scrA", name=f"velt{hf}{k}")
                    eng.tensor_scalar_mul(out=tv[:], in0=xT[:, k, hsl], scalar1=4097.0)
                    uv = scr.tile([128, 512], FP, tag="scrB", name=f"veltu{hf}{k}")
                    eng.tensor_tensor(
                        out=uv[:], in0=tv[:], in1=xT[:, k, hsl], op=A.subtract
                    )
                    eng.tensor_tensor(
                        out=xh_t[:, k, hsl], in0=tv[:], in1=uv[:], op=A.subtract
                    )
                    eng.tensor_tensor(
                        out=xl_t[:, k, hsl], in0=xT[:, k, hsl],
                        in1=xh_t[:, k, hsl].bitcast(FP), op=A.subtract,
                    )

            def xTr(k, sl):
                return xh_t[:, k, sl]

            # colsum(x) for the eps numerator term, emitted here so xT's
            # SBUF slot frees early for the q/k evac staging
            xsum = consts.tile([128, 2], FP, tag="xsum")
            for k in range(2):
                nc.vector.tensor_reduce(
                    out=xsum[:, k : k + 1], in_=xT[:, k, :], axis=AX.X, op=A.add
                )

            # hi/lo -> augR2 rows [-hi/2, -lo/2] (one transpose, split DMAs)
            pthl = psS.tile([128, C], FP, tag="ps_small", name="pthl")
            nc.tensor.transpose(out=pthl[0:16, 0:128], in_=hilo[:], identity=ident[:])
            t16 = scr.tile([32, 128], FR, tag="scrT", name="t16")
            nc.scalar.copy(out=t16[0:16, :], in_=pthl[0:16, 0:128])
            augL2 = consts.tile([2, N], FR, tag="augL2")
            augR2 = consts.tile([2, N], FR, tag="augR2")
            nc.gpsimd.memset(augL2[:].bitcast(FP), 1.0)
            half = NBLK // 2
            nc.sync.dma_start(out=augR2[0:1, 0 : N // 2], in_=t16[0:half, :])
            nc.sync.dma_start(out=augR2[0:1, N // 2 : N], in_=t16[half:NBLK, :])
            nc.sync.dma_start(
                out=augR2[1:2, 0 : N // 2], in_=t16[NBLK : NBLK + half, :]
            )
            nc.sync.dma_start(
                out=augR2[1:2, N // 2 : N], in_=t16[NBLK + half : 16, :]
            )

            # ---------------- d2 matmuls -> z (all f32r) ----------------
            # z is symmetric: compute only the upper-triangular slab per
            # row-block (columns >= 128*ib) and mirror the lower blocks via
            # PE transposes of already-evacuated rows. Chunks stay >=256
            # wide so f32r runs at 1 cyc/row (the 128-wide diagonal chunk of
            # the last row pays the narrow penalty once).
            def _chunks(off):
                # matmul output must stay inside one 512-col PSUM bank
                cl = []
                for b0, b1 in ((0, 512), (512, N)):
                    lo = max(off, b0)
                    if lo < b1:
                        cl.append(slice(lo, b1))
                return cl

            zt = big.tile([128, NBLK, N], FP, tag="zbig")
            for ib in range(NBLK):
                pd = psA.tile([128, N], FP, tag="ps_big")
                ibs = slice(128 * ib, 128 * (ib + 1))
                for sl in _chunks(128 * ib):
                    for k in range(2):
                        for lhs, rhs in (
                            (xh_t, xh_t), (xh_t, xl_t), (xl_t, xh_t)
                        ):
                            nc.tensor.matmul(
                                pd[:, sl],
                                lhs[:, k, ibs],
                                rhs[:, k, sl],
                                start=(k == 0 and lhs is xh_t and rhs is xh_t),
                                stop=False,
                            )
                    nc.tensor.matmul(
                        pd[:, sl],
                        augL2[:, ibs],
                        augR2[:, sl],
                        start=False,
                        stop=True,
                    )
                # mirrors: z[ib-block rows, jb cols] = transpose of the
                # (jb, ib) upper block evacuated in an earlier iteration
                for jb in range(ib):
                    ptm = psS.tile(
                        [128, 128], FP, tag="ps_small", name=f"mir{ib}_{jb}"
                    )
                    nc.tensor.transpose(
                        out=ptm[:], in_=zt[:, jb, ibs], identity=ident[:]
                    )
                    nc.scalar.copy(
                        out=zt[:, ib, 128 * jb : 128 * (jb + 1)], in_=ptm[:]
                    )
                # psum = x.x - sq_j/2 -> z = psum/128 + (C0 - sq_i/256)
                nc.scalar.activation(
                    out=zt[:, ib, 128 * ib : N], in_=pd[:, 128 * ib : N],
                    func=AF.Identity,
                    bias=zbias_col[:, ib : ib + 1], scale=1.0 / 128.0,
                )

            # ================= clustering =================
            # density + its exact block-broadcast run PER BLOCK: block b only
            # needs d2 block b, so all of this hides under the d2 phase and
            # densb is complete ~1 block after the last z evac
            z5 = mid.tile([128, NBLK, 8], FP, tag="z5")
            sum5 = mid.tile([128, NBLK], FP, tag="sum5")
            dens_col = mid.tile([128, NBLK], FP, tag="dens_col")
            negc0 = mid.tile([128, 1], FP, tag="negc0")
            nc.vector.memset(negc0[:], -C0)
            densb = mid.tile([128, N], FP, tag="densb")
            for b in range(NBLK):
                nc.vector.max(out=z5[:, b, :], in_=zt[:, b, :])
                nc.vector.tensor_reduce(
                    out=sum5[:, b : b + 1], in_=z5[:, b, 0:K5], axis=AX.X, op=A.add
                )
                nc.scalar.activation(
                    out=dens_col[:, b : b + 1], in_=sum5[:, b : b + 1],
                    func=AF.Exp, bias=negc0[:], scale=1.0 / K5,
                )
                nc.vector.tensor_add(
                    out=dens_col[:, b : b + 1], in0=dens_col[:, b : b + 1],
                    in1=noise_sb[:, b : b + 1],
                )
                stage = scr.tile([128, 128], FP, tag="scrT", name=f"dstg{b}")
                nc.scalar.activation(
                    out=stage[:], in_=ident[:], func=AF.Identity,
                    bias=dens_col[:, b : b + 1], scale=0.0,
                )
                ptd = psS.tile([128, 128], FP, tag="ps_small", name=f"dpt{b}")
                nc.tensor.transpose(out=ptd[:], in_=stage[:], identity=ident[:])
                nc.scalar.copy(
                    out=densb[:, 128 * b : 128 * (b + 1)], in_=ptd[:]
                )

            # d_ind^2 = C0 - max over {j: dens_j > dens_i} of z_ij
            # Blocks 0-2: Pool builds gt-mask + masked product ({0,1} mask,
            # exact), DVE only reduces. Blocks 3-7: all-DVE. score2 and its
            # exact broadcast trail each block so rank starts right after.
            u_col = mid.tile([128, NBLK], FP, tag="u_col")
            negdens2_col = mid.tile([128, NBLK], FP, tag="dens2_col")
            nc.vector.scalar_tensor_tensor(
                out=negdens2_col[:], in0=dens_col[:], scalar=-1.0,
                in1=dens_col[:], op0=A.mult, op1=A.mult,
            )
            score2_col = mid.tile([128, NBLK], FP, tag="score2_col")
            score2b = mid.tile([128, N], FP, tag="score2b")
            # Column halves: the first-half masked-max only needs densb cols
            # 0:512 (density blocks 0-3) and z columns 0:512, both ready
            # while d2 is still streaming -> half 1 hides in d2's DVE gaps.
            # DVE masks blocks 0-4, Pool builds masked products for 5-7.
            u1_col = mid.tile([128, NBLK], FP, tag="u1_col")
            for hf3 in range(2):
                hsl3 = slice(512 * hf3, 512 * (hf3 + 1))
                for b in range(NBLK):
                    tagp = ("scrA", "scrB")[(b + hf3) % 2]
                    tagg = ("scrB", "scrA")[(b + hf3) % 2]
                    prod = scr.tile([128, 512], FP, tag=tagp, name=f"up{hf3}{b}")
                    if b >= 5:
                        gb = scr.tile(
                            [128, 512], FP, tag=tagg, name=f"gb{hf3}{b}"
                        )
                        nc.gpsimd.tensor_scalar(
                            out=gb[:], in0=densb[:, hsl3],
                            scalar1=dens_col[:, b : b + 1],
                            scalar2=None, op0=A.is_gt,
                        )
                        nc.gpsimd.tensor_tensor(
                            out=prod[:], in0=zt[:, b, hsl3], in1=gb[:],
                            op=A.mult,
                        )
                    else:
                        nc.vector.scalar_tensor_tensor(
                            out=prod[:],
                            in0=densb[:, hsl3],
                            scalar=dens_col[:, b : b + 1],
                            in1=zt[:, b, hsl3],
                            op0=A.is_gt,
                            op1=A.mult,
                        )
                    udst = u1_col if hf3 == 0 else u_col
                    nc.vector.tensor_reduce(
                        out=udst[:, b : b + 1], in_=prod[:], axis=AX.X,
                        op=A.max,
                    )
                    if hf3 == 0:
                        continue
                    nc.vector.tensor_max(
                        out=u_col[:, b : b + 1], in0=u_col[:, b : b + 1],
                        in1=u1_col[:, b : b + 1],
                    )
                    # score2 = (u - C0) * (-dens^2) = (C0 - u) * dens^2
                    nc.vector.scalar_tensor_tensor(
                        out=score2_col[:, b : b + 1], in0=u_col[:, b : b + 1],
                        scalar=C0, in1=negdens2_col[:, b : b + 1],
                        op0=A.subtract, op1=A.mult,
                    )
                    stg2 = scr.tile([128, 128], FP, tag="scrT", name=f"sstg{b}")
                    nc.scalar.activation(
                        out=stg2[:], in_=ident[:], func=AF.Identity,
                        bias=score2_col[:, b : b + 1], scale=0.0,
                    )
                    pts = psS.tile([128, 128], FP, tag="ps_small", name=f"spt{b}")
                    nc.tensor.transpose(
                        out=pts[:], in_=stg2[:], identity=ident[:]
                    )
                    nc.scalar.copy(
                        out=score2b[:, 128 * b : 128 * (b + 1)], in_=pts[:]
                    )

            # rank in column halves: first-half compares start as soon as
            # score2b blocks 0-3 are broadcast, overlapping the rest
            rank_col = mid.tile([128, NBLK], FP, tag="rank_col")
            rank2_col = mid.tile([128, NBLK], FP, tag="rank2_col")
            for hf2 in range(2):
                hsl2 = slice(512 * hf2, 512 * (hf2 + 1))
                rdst = rank_col if hf2 == 0 else rank2_col
                for b in range(NBLK):
                    rsc = scr.tile(
                        [128, 512], BF, tag="scrA" if hf2 == 0 else "scrB",
                        name=f"rsc{hf2}{b}",
                    )
                    nc.vector.tensor_scalar(
                        out=rsc[:],
                        in0=score2b[:, hsl2],
                        scalar1=score2_col[:, b : b + 1],
                        scalar2=None,
                        op0=A.is_gt,
                        op1=A.add,
                        accum_out=rdst[:, b : b + 1],
                    )
            nc.vector.tensor_add(
                out=rank_col[:], in0=rank_col[:], in1=rank2_col[:]
            )
            # cm = rank < 255.5 (top-256 by score), in column AND row form
            cm_col = mid.tile([128, NBLK], FP, tag="cm_col")
            nc.vector.tensor_scalar(
                out=cm_col[:], in0=rank_col[:], scalar1=float(256) - 0.5,
                scalar2=None, op0=A.is_lt,
            )
            # cmb values are exactly {0,1}, so the cheap f32r PE replicate is
            # exact (blockbcast is only needed for arbitrary fp32 rows).
            # Cluster ids are the raw center column index j* (0..1023), so no
            # prefix-sum renumbering (crank/pack) is needed at all: the
            # argmax column index IS the id, split as 32a+b with a,b in
            # 0..31 -> 64 one-hot mask rows + 64 head dims = 128 rows.
            # cmb per block via stage+transpose (no [1,N] row trip, no PE
            # replicate): ACT stage while DVE evacuates, ~3 blocks in flight
            cmb = mid.tile([128, N], FP, tag="densb", name="cmb")
            for b in range(NBLK):
                stg3 = scr.tile([128, 128], FP, tag="scrT", name=f"cmstg{b}")
                nc.scalar.activation(
                    out=stg3[:], in_=ident[:], func=AF.Identity,
                    bias=cm_col[:, b : b + 1], scale=0.0,
                )
                ptc = psS.tile([128, 128], FP, tag="ps_small", name=f"cmpt{b}")
                nc.tensor.transpose(out=ptc[:], in_=stg3[:], identity=ident[:])
                nc.vector.tensor_scalar_add(
                    out=cmb[:, 128 * b : 128 * (b + 1)], in0=ptc[:], scalar1=0.0
                )

            # ------- assignment: v_i = argmax_{centers j} z_ij (raw col) ----
            # Pool masks non-centers to 0 (centers keep z >= ~4 > 0), DVE
            # max + max_index return the winning column index directly
            # (uint32, lowest-index tie-break like the fp32 argmin).
            U32 = mybir.dt.uint32
            vmax8 = mid.tile([128, NBLK, 8], FP, tag="rmax_col", name="vmax8")
            vidx8 = mid.tile([128, NBLK, 8], U32, tag="v_col", name="vidx8")
            for b in range(NBLK):
                # alternate scratch tags -> 4 m2 slots, so Pool streams the
                # masks back-to-back instead of waiting on DVE's max_index
                m2 = scr.tile(
                    [128, N], FP, tag="scrA" if b % 2 == 0 else "scrB",
                    name=f"m2{b}",
                )
                nc.gpsimd.tensor_tensor(
                    out=m2[:], in0=zt[:, b, :], in1=cmb[:], op=A.mult
                )
                nc.vector.max(out=vmax8[:, b, :], in_=m2[:])
                nc.vector.max_index(
                    out=vidx8[:, b, :], in_max=vmax8[:, b, :], in_values=m2[:]
                )
            vf_col = mid.tile([128, NBLK], FP, tag="vf_col")
            nc.vector.tensor_copy(out=vf_col[:], in_=vidx8[:, :, 0])
            # ------- W transposes + q/k/v/colsum fill the c-stage PE idle ----
            wT = {}
            for nm in ("q", "k", "v", "p"):
                wt = consts.tile([128, 2, C], FR, tag=f"wT{nm}", name=f"wT{nm}")
                for a in range(2):
                    for b2 in range(2):
                        pt = psS.tile([128, 128], FP, tag="ps_small")
                        nc.tensor.transpose(
                            out=pt[:],
                            in_=wraw[nm][:, a, 128 * b2 : 128 * (b2 + 1)],
                            identity=ident[:],
                        )
                        nc.scalar.mul(
                            out=wt[:, b2, 128 * a : 128 * (a + 1)],
                            in_=pt[:],
                            mul=0.125 if nm == "q" else 1.0,
                        )
                wT[nm] = wt

            # rows 0:32 a-digit one-hot, 32:64 b-digit one-hot, 64:128 head
            # dims; the -256 constant is folded into the exp evac bias
            qTm = [consts.tile([128, N], FR, tag=f"qTm{h}", name=f"qTm{h}") for h in range(H)]
            kTm = [consts.tile([128, N], FR, tag=f"kTm{h}", name=f"kTm{h}") for h in range(H)]
            for dsts, wtile in ((qTm, wT["q"]), (kTm, wT["k"])):
                for m in range(2):
                    pq = psA.tile([128, N], FP, tag="ps_big")
                    for n_ in range(2):
                        sl = slice(512 * n_, 512 * (n_ + 1))
                        for k in range(2):
                            nc.tensor.matmul(
                                pq[:, sl],
                                wtile[:, k, 128 * m : 128 * (m + 1)],
                                xTr(k, sl),
                                start=(k == 0),
                                stop=(k == 1),
                            )
                    # evac into the dead xT slot (xsum hoisted earlier), so
                    # the scr pool stays free for the assignment pipeline
                    tqk = consts.tile([128, N], FR, tag="xT", name=f"tqk{m}")
                    nc.scalar.copy(out=tqk[:], in_=pq[:])
                    nc.sync.dma_start(out=dsts[2 * m][64:128, :], in_=tqk[0:64, :])
                    nc.sync.dma_start(
                        out=dsts[2 * m + 1][64:128, :], in_=tqk[64:128, :]
                    )

            va = consts.tile([128, NBLK, H, 65], FR, tag="va")
            for jb in range(NBLK):
                pv = psS.tile([128, C], FP, tag="ps_small")
                for k in range(2):
                    nc.tensor.matmul(
                        pv[:],
                        xTr(k, slice(128 * jb, 128 * (jb + 1))),
                        wT["v"][:, k, :],
                        start=(k == 0),
                        stop=(k == 1),
                    )
                nc.scalar.copy(
                    out=va[:, jb, :, 0:64],
                    in_=pv[:].rearrange("p (h d) -> p h d", h=H),
                )
                nc.vector.memset(va[:, jb, :, 64:65].bitcast(FP), 1.0)

            wpproj = consts.tile([64, H, C], FR, tag="wpproj")
            for h in range(H):
                nc.sync.dma_start(
                    out=wpproj[:, h, :],
                    in_=wT["p"][64 * (h % 2) : 64 * (h % 2) + 64, h // 2, :],
                )

            cs_sb = consts.tile([64, H], FP, tag="cs_sb")
            for m in range(2):
                pc = psS.tile([128, C], FP, tag="ps_small")
                for k in range(2):
                    nc.tensor.matmul(
                        pc[:, 0:1],
                        wT["v"][:, k, 128 * m : 128 * (m + 1)].bitcast(FP),
                        xsum[:, k : k + 1],
                        start=(k == 0),
                        stop=(k == 1),
                    )
                tpc = scr.tile([128, 1], FP, tag="scrT", name="tpc")
                nc.scalar.copy(out=tpc[:], in_=pc[:, 0:1])
                for hh in range(2):
                    nc.sync.dma_start(
                        out=cs_sb[:, 2 * m + hh : 2 * m + hh + 1],
                        in_=tpc[64 * hh : 64 * hh + 64, :],
                    )
            nc.vector.tensor_scalar_mul(out=cs_sb[:], in0=cs_sb[:], scalar1=EPS / N)

            # digits in COLUMN space: a' = round((v+16.5)/32) = floor(v/32)+1
            # via the round-to-nearest f32->u32 convert (b-15.5 stays within
            # +-0.48 of 0), b' = v - 32a' = b - 32; the one-hot iota values
            # absorb both offsets (a-rows compare 1..32, b-rows -32..-1).
            abf = mid.tile([128, 2, NBLK], FP, tag="abf")
            nc.vector.tensor_scalar(
                out=abf[:, 0, :], in0=vf_col[:], scalar1=16.5,
                scalar2=1.0 / 32.0, op0=A.add, op1=A.mult,
            )
            au_col = mid.tile([128, NBLK], U32, tag="au_col")
            nc.vector.tensor_copy(out=au_col[:], in_=abf[:, 0, :])
            nc.vector.tensor_copy(out=abf[:, 0, :], in_=au_col[:])
            nc.vector.scalar_tensor_tensor(
                out=abf[:, 1, :], in0=abf[:, 0, :], scalar=-32.0,
                in1=vf_col[:], op0=A.mult, op1=A.add,
            )
            # one transpose carries both digit rows; queue-parallel DMAs
            ptab = psS.tile([128, C], FP, tag="ps_small", name="ptab")
            nc.tensor.transpose(
                out=ptab[0:16, 0:128], in_=abf[:].rearrange("p a b -> p (a b)"),
                identity=ident[:],
            )
            tab = scr.tile([32, 128], FR, tag="scrT", name="tab")
            nc.scalar.copy(out=tab[0:16, :], in_=ptab[0:16, 0:128])
            # digits -> [64,N] via one selector matmul per block straight
            # from tab (no [1,N] row round-trip through DMA): out block b
            # rows 0:32 pick tab row b (a'), rows 32:64 pick row 8+b (b')
            pC = psA.tile([128, N], FP, tag="ps_big", name="pC")
            for b in range(NBLK):
                nc.tensor.matmul(
                    pC[0:64, 128 * b : 128 * (b + 1)],
                    selblk[:, 64 * b : 64 * (b + 1)],
                    tab[0:16, :],
                    start=True, stop=True,
                )
            # head-0 mask rows read the PSUM directly (scores start at once);
            # heads 1..3 copied in the background by DMA
            nc.vector.tensor_scalar(
                out=qTm[0][0:64, :], in0=pC[0:64, :],
                scalar1=iota64[:, 0:1], scalar2=None, op0=A.is_equal,
            )
            nc.vector.tensor_scalar(
                out=kTm[0][0:64, :], in0=pC[0:64, :],
                scalar1=iota64[:, 0:1], scalar2=BIGM, op0=A.is_equal, op1=A.mult,
            )
            for h in range(1, H):
                nc.sync.dma_start(out=qTm[h][0:64, :], in_=qTm[0][0:64, :])
                nc.sync.dma_start(out=kTm[h][0:64, :], in_=kTm[0][0:64, :])

            # ================= attention =================
            PT = big.tile([128, NBLK, N], FR, tag="zbig")  # reuses z slot
            outTs = [mid.tile([65, N], FP, tag=f"outT{h % 2}", name=f"outT{h}")
                     for h in range(H)]
            numT = [mid.tile([64, N], FR, tag=f"numT{h}", name=f"numT{h}") for h in range(H)]
            recips = [mid.tile([1, N], FR, tag=f"recip{h % 2}", name=f"recip{h}")
                      for h in range(H)]

            def head_tail(h):
                # (outT + eps_colsum) / den, in column halves; psS psum for
                # the broadcast (psPV would collide with the next head's PV
                # accumulator, bufs=1); numT reads the broadcast from PSUM
                # directly, no SBUF evac.
                for hf in range(2):
                    sl = slice(512 * hf, 512 * (hf + 1))
                    pbh = psS.tile(
                        [64, 512], FP, tag="ps_small", name=f"ht{h}{hf}"
                    )
                    nc.tensor.matmul(
                        pbh[:], ones1r[:, 0:64], recips[h][0:1, sl],
                        start=True, stop=True,
                    )
                    nc.vector.scalar_tensor_tensor(
                        out=numT[h][:, sl],
                        in0=outTs[h][0:64, sl],
                        scalar=cs_sb[:, h : h + 1],
                        in1=pbh[:],
                        op0=A.add,
                        op1=A.mult,
                    )

            for h in range(H):
                for jb in range(NBLK):
                    pst = psA.tile([128, N], FP, tag="ps_big")
                    for n_ in range(2):
                        sl = slice(512 * n_, 512 * (n_ + 1))
                        nc.tensor.matmul(
                            pst[:, sl],
                            kTm[h][:, 128 * jb : 128 * (jb + 1)],
                            qTm[h][:, sl],
                            start=True,
                            stop=True,
                        )
                    nc.scalar.activation(
                        out=PT[:, jb, :], in_=pst[:], func=AF.Exp,
                        bias=negbig[:, 0:1], scale=1.0,
                    )
                if h == H - 1:
                    # h2's tail deps are ready mid-score-phase; emit before
                    # the PV loop so its PE/DVE ops clear before ppv lands
                    head_tail(h - 1)
                ppv = psPV.tile([65, N], FP, tag="ps_pv")
                for jb in range(NBLK):
                    for n_ in range(2):
                        sl = slice(512 * n_, 512 * (n_ + 1))
                        nc.tensor.matmul(
                            ppv[:, sl],
                            va[:, jb, h, :],
                            PT[:, jb, sl],
                            start=(jb == 0),
                            stop=(jb == NBLK - 1),
                        )
                if h == 0:
                    mkrow("den")
                if h < H - 1:
                    nc.vector.tensor_scalar_add(
                        out=outTs[h][:], in0=ppv[:], scalar1=0.0
                    )
                    nc.vector.tensor_scalar(
                        out=row("den"), in0=outTs[h][64:65, :], scalar1=EPS,
                        scalar2=None, op0=A.add,
                    )
                    with nc.allow_low_precision(reason="f32r recip -> f32r bc"):
                        nc.vector.reciprocal(out=recips[h][0:1, :], in_=row("den"))
                    if h > 0:
                        head_tail(h - 1)
                else:
                    # last head: its tail is fully exposed, so pipeline the
                    # chain in column halves; den/recip read the PV PSUM row
                    # directly while the outT evac runs on now-idle ACT
                    for hf in range(2):
                        sl = slice(512 * hf, 512 * (hf + 1))
                        nc.scalar.copy(
                            out=outTs[h][:, sl], in_=ppv[:, sl]
                        )
                        nc.vector.tensor_scalar(
                            out=p0rows["den"][0:1, sl],
                            in0=ppv[64:65, sl], scalar1=EPS,
                            scalar2=None, op0=A.add,
                        )
                        with nc.allow_low_precision(reason="f32r recip bc"):
                            nc.vector.reciprocal(
                                out=recips[h][0:1, sl],
                                in_=p0rows["den"][0:1, sl],
                            )
                        pbh = psS.tile([64, 512], FP, tag="ps_small",
                                       name=f"pbh{hf}")
                        nc.tensor.matmul(
                            pbh[:], ones1r[:, 0:64], recips[h][0:1, sl],
                            start=True, stop=True,
                        )
                        nc.vector.scalar_tensor_tensor(
                            out=numT[h][:, sl],
                            in0=outTs[h][0:64, sl],
                            scalar=cs_sb[:, h : h + 1],
                            in1=pbh[:],
                            op0=A.add,
                            op1=A.mult,
                        )

            # ---------------- output projection ----------------
            # yout holds all 8 blocks so no evac ever waits on a DMA
            # completing (the old 2-slot scratch serialized the tail).
            out_r = out_ext.rearrange("(b p) c -> p b c", p=128)
            # two 4-block staging tiles on dead tags: no evac ever waits on
            # an output DMA completing
            yout_lo = mid.tile([128, 4, C], FP, tag="densb", name="yout_lo3")
            yout_hi = mid.tile([128, 4, C], FP, tag="outT0", name="yout_hi")
            for ib in range(NBLK):
                py = psS.tile([128, C], FP, tag="ps_small")
                for h in range(H):
                    nc.tensor.matmul(
                        py[:],
                        numT[h][:, 128 * ib : 128 * (ib + 1)],
                        wpproj[:, h, :],
                        start=(h == 0),
                        stop=False,
                    )
                nc.tensor.matmul(
                    py[:], ones1r[:], bp_fr[:], start=False, stop=True
                )
                yo = yout_lo if ib < 4 else yout_hi
                nc.scalar.copy(out=yo[:, ib % 4, :], in_=py[:])
                if ib % 2 == 1:
                    s4 = (ib - 1) % 4
                    nc.sync.dma_start(
                        out=out_r[:, ib - 1 : ib + 1, :],
                        in_=yo[:, s4 : s4 + 2, :],
                    )

    return nc


_CACHE = {}


def _get_nc():
    if "nc" not in _CACHE:
        _CACHE["nc"] = build_nc()
        _CACHE["noise"] = _noise_cols()
        _CACHE["ident"] = np.eye(128, dtype=np.float32)
        _CACHE["iota64"] = np.concatenate(
            [np.arange(32) + 1, np.arange(32) - 32]
        ).reshape(64, 1).astype(np.float32)
        selblk = np.zeros((16, 512), dtype=np.float32)
        for _b in range(8):
            selblk[_b, 64 * _b : 64 * _b + 32] = 1.0
            selblk[8 + _b, 64 * _b + 32 : 64 * _b + 64] = 1.0
        _CACHE["selblk"] = selblk
    return _CACHE["nc"]


def kernel(x_token, Wq, Wk, Wv, Wp, bp, _trace=False, _trace_kwargs=None):
    from concourse.bass_utils import run_bass_kernel_spmd

    nc = _get_nc()
    noise = _CACHE["noise"]
    x_token = np.ascontiguousarray(np.asarray(x_token, dtype=np.float32))
    weights = {
        "Wq": np.ascontiguousarray(np.asarray(Wq, dtype=np.float32)),
        "Wk": np.ascontiguousarray(np.asarray(Wk, dtype=np.float32)),
        "Wv": np.ascontiguousarray(np.asarray(Wv, dtype=np.float32)),
        "Wp": np.ascontiguousarray(np.asarray(Wp, dtype=np.float32)),
        "bp": np.ascontiguousarray(np.asarray(bp, dtype=np.float32)),
    }
    consts = {
        "ident": _CACHE["ident"],
        "iota64": _CACHE["iota64"],
        "selblk": _CACHE["selblk"],
    }
    in_maps = []
    for b in range(B):
        in_maps.append(dict(weights, x=x_token[b], noise=noise[b], **consts))
    kw = {}
    if _trace:
        kw = dict(trace=True, trace_kwargs=_trace_kwargs or {})
    res = run_bass_kernel_spmd(nc, in_maps, list(range(B)), **kw)
    out = np.stack([res.results[b]["out"] for b in range(B)], axis=0)
    if _trace:
        return out, res
    return out



# revision 97
# speedup vs baseline: 1.0338x; 1.0338x over previous
"""Trainium2 Bass kernel for nn_CTAttention2 (DPC-KNN cluster attention).

Self-contained: accepts FULL inputs (B=8,N=1024,C=256), shards batch across
8 NeuronCores (one batch element per core), runs a fully fused Bass/Tile
kernel per core, and gathers the full output.

v2 redesign vs the fp32 baseline (272us): every large matmul now runs in
f32r (1 cycle/row vs fp32's 4) and the same-cluster attention mask is
restructured so it stays EXACT in any PE precision:

  - cluster ids c0 in 0..255 are split into digits a=c0/16, b=c0%16 and
    encoded as 32 one-hot rows (products only 0/1/128/-256, exactly
    representable).  kTm/qTm are [97,N]: rows 0..31 one-hot (k side x128,
    q side x1), row 32 the -256 constant, rows 33..96 the head dims.  A
    same-cluster pair accumulates +128+128-256 = 0 exactly; a distinct
    pair is <= -128 so exp() flushes to ~0 (vs reference's exp(-1e9)).
  - d2 matmuls run on f32r with a hi/lo split of sq (granularity 2^-2 via
    the +-2^21 trick) across four augmentation rows, so the sq terms stay
    exact and only the x.x products see f32r rounding (~1e-4 abs on d2).
  - digit rows are recovered without mod/floor (unsupported ALU ops):
    pack = 48*a + c0 = 64a+b is gathered per token by the assignment
    scatter, then a = sum of 15 range compares (PE partition-sum), and
    b = pack - 64a.
  - density/score2 compare broadcasts stay exact fp32 (a rounded broadcast
    would corrupt self-compares in the is_gt masks); {0,1}/integer/recip
    broadcasts use f32r ones (1 cyc/row).
  - reduces and plain tensor_tensor ops move to the Pool engine; PSUM
    evacuations are split between Act and DVE to keep the score-exp loop
    (Act-bound) and the clustering chain (DVE-bound) short.
"""

import os
import sys

for _p in ("/opt/trn_rl_repo", "/root/.axon_site/_ro/trn_rl_repo"):
    if os.path.isdir(_p) and _p not in sys.path:
        sys.path.insert(0, _p)

import numpy as np

import concourse.bass as bass
import concourse.tile as tile
from concourse import mybir
from concourse.bass_types import SemaphoreHandle
from concourse.vector_clock import ScopedClock

B, N, C = 8, 1024, 256
H, D = 4, 64
NBLK = N // 128
K5 = 5
EPS = 1e-6
C0 = 8.0          # z = C0 - d2/256
BIGM = 128.0      # one-hot mask weight; exp(s - 128) == 0.0 for |s|<=~20
SPLIT = float(2 ** 21)  # hi/lo split constant for sq (granularity 0.25)
FP = mybir.dt.float32
FR = mybir.dt.float32r
BF = mybir.dt.bfloat16
A = mybir.AluOpType
AF = mybir.ActivationFunctionType
AX = mybir.AxisListType


# ---------------------------------------------------------------------------
# Workaround: this walrus build rejects the multi-wait tail Drain emitted by
# TileContext ("Too many sync wait commands").  Emit one single-wait SP
# instruction per outstanding semaphore instead, then a wait-free drain.
# ---------------------------------------------------------------------------
def _patched_drain_and_barrier(self, tick_clock, wait_clock):
    nc = self.nc
    probe = mybir.InstNoOp(name=f"drain-probe-{nc.next_id()}", ins=[], outs=[])
    probe.engine = mybir.EngineType.SP
    wait_clock.add_sem_waits(probe, ScopedClock({None: tick_clock.global_clock}))
    if probe.sync_info is not None:
        for w in probe.sync_info.on_wait:
            assert w.wait_mode == "sem-ge-imm", w
            nc.sync.wait_ge(SemaphoreHandle(w.ant_name, w.id), w.wait_value)
    nc.sync.drain()
    nc.all_engine_barrier()
    popped = nc._tile_sem_poison_stack.pop()
    assert popped is self._sem_poison
    nc.clear_and_free_semaphores(list(self.sems.allocated().values()))
    nc.all_engine_barrier()


def _install_drain_patch():
    tile.TileContext._drain_and_barrier = _patched_drain_and_barrier


# ---------------------------------------------------------------------------
# Workaround #2: the same walrus build caps the number of sync-wait commands
# per instruction.  Post-process the BIR JSON just before the walrus call:
# move excess waits onto single-wait NoOps inserted immediately before the
# instruction on the same engine.
# ---------------------------------------------------------------------------
_WAIT_CAPS = {"default": 1}


def _split_excess_waits(bir_json):
    import json as _json

    d = _json.loads(bir_json)
    changed = False
    for fn in d.get("functions", []):
        for bb in fn.get("blocks", []):
            out = []
            for inst in bb.get("instructions", []):
                si = inst.get("sync_info")
                waits = (si or {}).get("on_wait") or []
                cap = _WAIT_CAPS.get(inst.get("opcode"), _WAIT_CAPS["default"])
                if len(waits) > cap:
                    keep = waits[-cap:] if cap > 0 else []
                    extra = waits[: len(waits) - cap]
                    for k, w in enumerate(extra):
                        carrier = {
                            "name": f"{inst['name']}__w{k}",
                            "opcode": "EventSemaphore",
                            "engine": inst["engine"],
                            "ins": [],
                            "outs": [],
                            "sync_info": {"on_wait": [w], "on_update": []},
                        }
                        if "debug" in inst:
                            carrier["debug"] = inst["debug"]
                        out.append(carrier)
                    si["on_wait"] = keep
                    changed = True
                out.append(inst)
            bb["instructions"] = out
    if not changed:
        return bir_json
    return _json.dumps(d).encode()


_ORIG_COMPILE = {}


def _install_wait_split_patch():
    import concourse.bass2jax as bass2jax
    import concourse.bass_utils as bass_utils

    if "impl" in _ORIG_COMPILE:
        return
    orig = bass_utils.compile_bir_kernel
    _ORIG_COMPILE["impl"] = orig

    def patched(bir_json, tmpdir, neff_name="file.neff"):
        return orig(_split_excess_waits(bir_json), tmpdir, neff_name=neff_name)

    bass_utils.compile_bir_kernel = patched
    bass2jax.compile_bir_kernel = patched


def _noise_cols():
    """Reference tie-break noise, per core, in [part, blk] layout, x1e-6."""
    import jax
    import jax.numpy as jnp

    with jax.default_device(jax.devices("cpu")[0]):
        u = jax.random.uniform(jax.random.key(42), (B, N), dtype=jnp.float32)
    u = np.asarray(u).astype(np.float32) * np.float32(1e-6)
    # token i = 128*blk + part  ->  [core][part, blk]
    return [np.ascontiguousarray(u[b].reshape(NBLK, 128).T) for b in range(B)]


def build_nc():
    _install_drain_patch()
    _install_wait_split_patch()
    nc = bass.Bass(num_swdge_queues=4)

    x_ext = nc.declare_dram_parameter("x", [N, C], FP, isOutput=False)
    wq_ext = nc.declare_dram_parameter("Wq", [C, C], FP, isOutput=False)
    wk_ext = nc.declare_dram_parameter("Wk", [C, C], FP, isOutput=False)
    wv_ext = nc.declare_dram_parameter("Wv", [C, C], FP, isOutput=False)
    wp_ext = nc.declare_dram_parameter("Wp", [C, C], FP, isOutput=False)
    bp_ext = nc.declare_dram_parameter("bp", [C], FP, isOutput=False)
    noise_ext = nc.declare_dram_parameter("noise", [128, NBLK], FP, isOutput=False)
    ident_ext = nc.declare_dram_parameter("ident", [128, 128], FP, isOutput=False)
    iota64_ext = nc.declare_dram_parameter("iota64", [64, 1], FP, isOutput=False)
    selblk_ext = nc.declare_dram_parameter("selblk", [16, 512], FP, isOutput=False)
    out_ext = nc.declare_dram_parameter("out", [N, C], FP, isOutput=True)

    wexts = {"q": wq_ext, "k": wk_ext, "v": wv_ext, "p": wp_ext}

    with tile.TileContext(nc) as tc:
        with (
            tc.tile_pool(name="consts", bufs=1) as consts,
            tc.tile_pool(name="big", bufs=1) as big,
            tc.tile_pool(name="mid", bufs=1) as mid,
            tc.tile_pool(name="scr", bufs=3) as scr,
            tc.tile_pool(name="psA", bufs=2, space="PSUM") as psA,
            tc.tile_pool(name="psS", bufs=2, space="PSUM") as psS,
            tc.tile_pool(name="psPV", bufs=1, space="PSUM") as psPV,
        ):
            # ---------------- loads ----------------
            xr = big.tile([128, NBLK, C], FP, tag="zbig")  # reused: xr -> z -> PT
            for bb2 in range(4):
                nc.sync.dma_start(
                    out=xr[:, 2 * bb2 : 2 * bb2 + 2, :],
                    in_=x_ext.rearrange("(b p) c -> p b c", p=128)[
                        :, 2 * bb2 : 2 * bb2 + 2, :
                    ],
                )
            ident = consts.tile([128, 128], FP, tag="ident")
            nc.sync.dma_start(out=ident[:], in_=ident_ext[:])
            noise_sb = consts.tile([128, NBLK], FP, tag="noise")
            nc.sync.dma_start(out=noise_sb[:], in_=noise_ext[:])
            bp_row = consts.tile([1, C], FP, tag="bp_row")
            nc.sync.dma_start(out=bp_row[:], in_=bp_ext.rearrange("(a c) -> a c", a=1))
            bp_fr = consts.tile([1, C], FR, tag="bp_fr")
            nc.scalar.copy(out=bp_fr[:], in_=bp_row[:])
            iota64 = consts.tile([64, 1], FP, tag="iota64")
            nc.sync.dma_start(out=iota64[:], in_=iota64_ext[:])
            selblkf = consts.tile([16, 512], FP, tag="selblkf")
            nc.sync.dma_start(out=selblkf[:], in_=selblk_ext[:])
            selblk = consts.tile([16, 512], FR, tag="selblk")
            nc.scalar.copy(out=selblk[:], in_=selblkf[:])

            wraw = {}
            for nm in ("q", "k", "v", "p"):
                t = consts.tile([128, 2, C], FP, tag=f"wraw{nm}", name=f"wraw{nm}")
                nc.sync.dma_start(
                    out=t[:], in_=wexts[nm].rearrange("(t p) c -> p t c", p=128)
                )
                wraw[nm] = t

            # ---------------- [1,N] row storage ----------------
            # all rows live at base partition 0 (engine ops require matching
            # start partitions); p0_* slots are reused across lifetimes.
            # Rows consumed by f32r matmuls are allocated as FR so their
            # producing instruction carries the f32r rounding flag.
            _P0TAG = {
                "brow": ("p0_1", FR),
                "arow": ("p0_3", FR),
                "den": ("p0_3", FP),
                "cm": ("p0_6", FP), "v": ("p0_6", FR),
            }
            p0rows = {}

            def mkrow(name):
                tag, dt = _P0TAG[name]
                p0rows[name] = mid.tile([1, N], dt, tag=tag, name=name)
                ap = p0rows[name][0:1, :]
                return ap if dt is FP else ap.bitcast(FP)

            def row(name):
                ap = p0rows[name][0:1, :]
                return ap if p0rows[name].dtype is FP else ap.bitcast(FP)

            def rowfr(name):
                return p0rows[name][0:1, :]

            ones1 = consts.tile([1, 128], FP, tag="ones1")
            nc.vector.memset(ones1[:], 1.0)
            negbig = consts.tile([128, 1], FP, tag="negbig")
            nc.vector.memset(negbig[:], -2.0 * BIGM)
            ones1r = consts.tile([1, 128], FR, tag="ones1r")
            nc.vector.memset(ones1r[:].bitcast(FP), 1.0)

            def col2row(dst, col_ap, nparts=NBLK, dt=FP):
                # [128, nparts] column tile -> [1, N] row: PE transpose,
                # evacuate (rounding to FR when the row feeds f32r matmuls),
                # then two queue-parallel gather DMAs.
                ptr = psS.tile([128, C], FP, tag="ps_small", name="ptr")
                nc.tensor.transpose(
                    out=ptr[0:nparts, 0:128], in_=col_ap, identity=ident[:]
                )
                t8 = scr.tile([32, 128], dt, tag="scrT", name="t8")
                nc.scalar.copy(out=t8[0:nparts, :], in_=ptr[0:nparts, 0:128])
                half = NBLK // 2
                nc.sync.dma_start(out=dst[:, 0 : N // 2], in_=t8[0:half, :])
                nc.sync.dma_start(out=dst[:, N // 2 : N], in_=t8[half:NBLK, :])

            def replicate(dst, src_row, parts=128, pool=None, evac="act"):
                # PE broadcast: f32r ones[1,parts] x row[1,N] -> psum -> SBUF.
                # f32r is exact for the {0,1}/small-integer/recip rows it is
                # used on here.
                nfree = dst.shape[-1]
                nsl = (nfree + 511) // 512
                pl = pool or psA
                pb = pl.tile(
                    [128, N], FP,
                    tag="ps_big" if pl is psA else "ps_pv", name="pb",
                )
                for n_ in range(nsl):
                    sl = slice(512 * n_, min(512 * (n_ + 1), nfree))
                    nc.tensor.matmul(
                        pb[0:parts, sl], ones1r[:, 0:parts],
                        src_row[:, sl], start=True, stop=True,
                    )
                if evac == "act":
                    nc.scalar.copy(out=dst[:], in_=pb[0:parts, 0:nfree])
                else:
                    nc.vector.tensor_scalar_add(
                        out=dst[:], in0=pb[0:parts, 0:nfree], scalar1=0.0
                    )

            def blockbcast(dst, col_ap_fn):
                # dst[p, 128*jb+jc] = col[jc, jb], exact fp32: per block an
                # Act bias-broadcast ([128,1] col -> [128,128]) + PE transpose.
                for jb in range(NBLK):
                    stage = scr.tile([128, 128], FP, tag="scrT", name="bbst")
                    nc.scalar.activation(
                        out=stage[:], in_=ident[:], func=AF.Identity,
                        bias=col_ap_fn(jb), scale=0.0,
                    )
                    pt = psS.tile([128, 128], FP, tag="ps_small")
                    nc.tensor.transpose(out=pt[:], in_=stage[:], identity=ident[:])
                    nc.scalar.copy(
                        out=dst[:, 128 * jb : 128 * (jb + 1)], in_=pt[:]
                    )

            # ---------------- sq = rowsum(x^2) (DVE, early) ----------------
            sq_col = consts.tile([128, NBLK], FP, tag="sq_col")
            for b in range(NBLK):
                scx = scr.tile([128, C], FP, tag="scrB")
                nc.vector.scalar_tensor_tensor(
                    out=scx[:],
                    in0=xr[:, b, :],
                    scalar=1.0,
                    in1=xr[:, b, :],
                    op0=A.mult,
                    op1=A.mult,
                    accum_out=sq_col[:, b : b + 1],
                )
            # hi/lo split of sq: hi = (sq + 2^21) - 2^21 (multiple of 0.25);
            # stored directly as -hi/2, -lo/2 column stacks for the j-side aug
            hilo = mid.tile([128, 16], FP, tag="hilo")
            nc.vector.tensor_scalar(
                out=hilo[:, 0:NBLK], in0=sq_col[:], scalar1=SPLIT, scalar2=-SPLIT,
                op0=A.add, op1=A.add,
            )
            nc.vector.tensor_tensor(
                out=hilo[:, NBLK:16], in0=sq_col[:], in1=hilo[:, 0:NBLK],
                op=A.subtract,
            )
            nc.vector.tensor_scalar_mul(out=hilo[:], in0=hilo[:], scalar1=-0.5)
            # z-evac bias: C0 - sq_i/256 per partition (exact fp32)
            zbias_col = mid.tile([128, NBLK], FP, tag="zbias_col")
            nc.vector.tensor_scalar(
                out=zbias_col[:], in0=sq_col[:], scalar1=-1.0 / 256.0, scalar2=C0,
                op0=A.mult, op1=A.add,
            )

            # ---------------- transposes (PE) ----------------
            xT = consts.tile([128, 2, N], FP, tag="xT")
            for hf in range(2):
                for t_ in range(2):
                    for r in range(4 * hf, 4 * hf + 4):
                        pt = psS.tile([128, 128], FP, tag="ps_small")
                        nc.tensor.transpose(
                            out=pt[:],
                            in_=xr[:, r, 128 * t_ : 128 * (t_ + 1)],
                            identity=ident[:],
                        )
                        nc.scalar.copy(
                            out=xT[:, t_, 128 * r : 128 * (r + 1)], in_=pt[:]
                        )

            # Veltkamp split at 12 bits: xh passes through f32r exactly,
            # xl carries the residual; x.x via 3 f32r matmuls is fp32-exact.
            # Chunked by column halves (half-major, all on DVE) so d2's first
            # 512-slice can start once cols 0:512 of both k-halves are split.
            xh_t = consts.tile([128, 2, N], FR, tag="xh")
            xl_t = consts.tile([128, 2, N], FR, tag="xl")
            for hf in range(2):
                hsl = slice(512 * hf, 512 * (hf + 1))
                for k in range(2):
                    # last chunk on Pool (idle in the prologue): DVE and Pool
                    # finish ~together so d2 streams without the hf=1 stall
                    eng = nc.vector if not (hf == 1 and k == 1) else nc.gpsimd
                    tv = scr.tile([128, 512], FP, tag="scrA", name=f"velt{hf}{k}")
                    eng.tensor_scalar_mul(out=tv[:], in0=xT[:, k, hsl], scalar1=4097.0)
                    uv = scr.tile([128, 512], FP, tag="scrB", name=f"veltu{hf}{k}")
                    eng.tensor_tensor(
                        out=uv[:], in0=tv[:], in1=xT[:, k, hsl], op=A.subtract
                    )
                    eng.tensor_tensor(
                        out=xh_t[:, k, hsl], in0=tv[:], in1=uv[:], op=A.subtract
                    )
                    eng.tensor_tensor(
                        out=xl_t[:, k, hsl], in0=xT[:, k, hsl],
                        in1=xh_t[:, k, hsl].bitcast(FP), op=A.subtract,
                    )

            def xTr(k, sl):
                return xh_t[:, k, sl]

            # colsum(x) for the eps numerator term, emitted here so xT's
            # SBUF slot frees early for the q/k evac staging
            xsum = consts.tile([128, 2], FP, tag="xsum")
            for k in range(2):
                nc.vector.tensor_reduce(
                    out=xsum[:, k : k + 1], in_=xT[:, k, :], axis=AX.X, op=A.add
                )

            # hi/lo -> augR2 rows [-hi/2, -lo/2] (one transpose, split DMAs)
            pthl = psS.tile([128, C], FP, tag="ps_small", name="pthl")
            nc.tensor.transpose(out=pthl[0:16, 0:128], in_=hilo[:], identity=ident[:])
            t16 = scr.tile([32, 128], FR, tag="scrT", name="t16")
            nc.scalar.copy(out=t16[0:16, :], in_=pthl[0:16, 0:128])
            augL2 = consts.tile([2, N], FR, tag="augL2")
            augR2 = consts.tile([2, N], FR, tag="augR2")
            nc.gpsimd.memset(augL2[:].bitcast(FP), 1.0)
            half = NBLK // 2
            nc.sync.dma_start(out=augR2[0:1, 0 : N // 2], in_=t16[0:half, :])
            nc.sync.dma_start(out=augR2[0:1, N // 2 : N], in_=t16[half:NBLK, :])
            nc.sync.dma_start(
                out=augR2[1:2, 0 : N // 2], in_=t16[NBLK : NBLK + half, :]
            )
            nc.sync.dma_start(
                out=augR2[1:2, N // 2 : N], in_=t16[NBLK + half : 16, :]
            )

            # ---------------- d2 matmuls -> z (all f32r) ----------------
            # z is symmetric: compute only the upper-triangular slab per
            # row-block (columns >= 128*ib) and mirror the lower blocks via
            # PE transposes of already-evacuated rows. Chunks stay >=256
            # wide so f32r runs at 1 cyc/row (the 128-wide diagonal chunk of
            # the last row pays the narrow penalty once).
            def _chunks(off):
                # matmul output must stay inside one 512-col PSUM bank
                cl = []
                for b0, b1 in ((0, 512), (512, N)):
                    lo = max(off, b0)
                    if lo < b1:
                        cl.append(slice(lo, b1))
                return cl

            zt = big.tile([128, NBLK, N], FP, tag="zbig")
            for ib in range(NBLK):
                pd = psA.tile([128, N], FP, tag="ps_big")
                ibs = slice(128 * ib, 128 * (ib + 1))
                for sl in _chunks(128 * ib):
                    for k in range(2):
                        for lhs, rhs in (
                            (xh_t, xh_t), (xh_t, xl_t), (xl_t, xh_t)
                        ):
                            nc.tensor.matmul(
                                pd[:, sl],
                                lhs[:, k, ibs],
                                rhs[:, k, sl],
                                start=(k == 0 and lhs is xh_t and rhs is xh_t),
                                stop=False,
                            )
                    nc.tensor.matmul(
                        pd[:, sl],
                        augL2[:, ibs],
                        augR2[:, sl],
                        start=False,
                        stop=True,
                    )
                # mirrors: z[ib-block rows, jb cols] = transpose of the
                # (jb, ib) upper block evacuated in an earlier iteration
                for jb in range(ib):
                    ptm = psS.tile(
                        [128, 128], FP, tag="ps_small", name=f"mir{ib}_{jb}"
                    )
                    nc.tensor.transpose(
                        out=ptm[:], in_=zt[:, jb, ibs], identity=ident[:]
                    )
                    nc.scalar.copy(
                        out=zt[:, ib, 128 * jb : 128 * (jb + 1)], in_=ptm[:]
                    )
                # psum = x.x - sq_j/2 -> z = psum/128 + (C0 - sq_i/256)
                nc.scalar.activation(
                    out=zt[:, ib, 128 * ib : N], in_=pd[:, 128 * ib : N],
                    func=AF.Identity,
                    bias=zbias_col[:, ib : ib + 1], scale=1.0 / 128.0,
                )

            # ================= clustering =================
            # density + its exact block-broadcast run PER BLOCK: block b only
            # needs d2 block b, so all of this hides under the d2 phase and
            # densb is complete ~1 block after the last z evac
            z5 = mid.tile([128, NBLK, 8], FP, tag="z5")
            sum5 = mid.tile([128, NBLK], FP, tag="sum5")
            dens_col = mid.tile([128, NBLK], FP, tag="dens_col")
            negc0 = mid.tile([128, 1], FP, tag="negc0")
            nc.vector.memset(negc0[:], -C0)
            densb = mid.tile([128, N], FP, tag="densb")
            for b in range(NBLK):
                nc.vector.max(out=z5[:, b, :], in_=zt[:, b, :])
                nc.vector.tensor_reduce(
                    out=sum5[:, b : b + 1], in_=z5[:, b, 0:K5], axis=AX.X, op=A.add
                )
                nc.scalar.activation(
                    out=dens_col[:, b : b + 1], in_=sum5[:, b : b + 1],
                    func=AF.Exp, bias=negc0[:], scale=1.0 / K5,
                )
                nc.vector.tensor_add(
                    out=dens_col[:, b : b + 1], in0=dens_col[:, b : b + 1],
                    in1=noise_sb[:, b : b + 1],
                )
                stage = scr.tile([128, 128], FP, tag="scrT", name=f"dstg{b}")
                nc.scalar.activation(
                    out=stage[:], in_=ident[:], func=AF.Identity,
                    bias=dens_col[:, b : b + 1], scale=0.0,
                )
                ptd = psS.tile([128, 128], FP, tag="ps_small", name=f"dpt{b}")
                nc.tensor.transpose(out=ptd[:], in_=stage[:], identity=ident[:])
                nc.scalar.copy(
                    out=densb[:, 128 * b : 128 * (b + 1)], in_=ptd[:]
                )

            # d_ind^2 = C0 - max over {j: dens_j > dens_i} of z_ij
            # Blocks 0-2: Pool builds gt-mask + masked product ({0,1} mask,
            # exact), DVE only reduces. Blocks 3-7: all-DVE. score2 and its
            # exact broadcast trail each block so rank starts right after.
            u_col = mid.tile([128, NBLK], FP, tag="u_col")
            negdens2_col = mid.tile([128, NBLK], FP, tag="dens2_col")
            nc.vector.scalar_tensor_tensor(
                out=negdens2_col[:], in0=dens_col[:], scalar=-1.0,
                in1=dens_col[:], op0=A.mult, op1=A.mult,
            )
            score2_col = mid.tile([128, NBLK], FP, tag="score2_col")
            score2b = mid.tile([128, N], FP, tag="score2b")
            # Column halves: the first-half masked-max only needs densb cols
            # 0:512 (density blocks 0-3) and z columns 0:512, both ready
            # while d2 is still streaming -> half 1 hides in d2's DVE gaps.
            # DVE masks blocks 0-4, Pool builds masked products for 5-7.
            u1_col = mid.tile([128, NBLK], FP, tag="u1_col")
            for hf3 in range(2):
                hsl3 = slice(512 * hf3, 512 * (hf3 + 1))
                for b in range(NBLK):
                    tagp = ("scrA", "scrB")[(b + hf3) % 2]
                    tagg = ("scrB", "scrA")[(b + hf3) % 2]
                    prod = scr.tile([128, 512], FP, tag=tagp, name=f"up{hf3}{b}")
                    if b >= 5:
                        gb = scr.tile(
                            [128, 512], FP, tag=tagg, name=f"gb{hf3}{b}"
                        )
                        nc.gpsimd.tensor_scalar(
                            out=gb[:], in0=densb[:, hsl3],
                            scalar1=dens_col[:, b : b + 1],
                            scalar2=None, op0=A.is_gt,
                        )
                        nc.gpsimd.tensor_tensor(
                            out=prod[:], in0=zt[:, b, hsl3], in1=gb[:],
                            op=A.mult,
                        )
                    else:
                        nc.vector.scalar_tensor_tensor(
                            out=prod[:],
                            in0=densb[:, hsl3],
                            scalar=dens_col[:, b : b + 1],
                            in1=zt[:, b, hsl3],
                            op0=A.is_gt,
                            op1=A.mult,
                        )
                    udst = u1_col if hf3 == 0 else u_col
                    nc.vector.tensor_reduce(
                        out=udst[:, b : b + 1], in_=prod[:], axis=AX.X,
                        op=A.max,
                    )
                    if hf3 == 0:
                        continue
                    nc.vector.tensor_max(
                        out=u_col[:, b : b + 1], in0=u_col[:, b : b + 1],
                        in1=u1_col[:, b : b + 1],
                    )
                    # score2 = (u - C0) * (-dens^2) = (C0 - u) * dens^2
                    nc.vector.scalar_tensor_tensor(
                        out=score2_col[:, b : b + 1], in0=u_col[:, b : b + 1],
                        scalar=C0, in1=negdens2_col[:, b : b + 1],
                        op0=A.subtract, op1=A.mult,
                    )
                    stg2 = scr.tile([128, 128], FP, tag="scrT", name=f"sstg{b}")
                    nc.scalar.activation(
                        out=stg2[:], in_=ident[:], func=AF.Identity,
                        bias=score2_col[:, b : b + 1], scale=0.0,
                    )
                    pts = psS.tile([128, 128], FP, tag="ps_small", name=f"spt{b}")
                    nc.tensor.transpose(
                        out=pts[:], in_=stg2[:], identity=ident[:]
                    )
                    nc.scalar.copy(
                        out=score2b[:, 128 * b : 128 * (b + 1)], in_=pts[:]
                    )

            # rank in column halves: first-half compares start as soon as
            # score2b blocks 0-3 are broadcast, overlapping the rest
            rank_col = mid.tile([128, NBLK], FP, tag="rank_col")
            rank2_col = mid.tile([128, NBLK], FP, tag="rank2_col")
            for hf2 in range(2):
                hsl2 = slice(512 * hf2, 512 * (hf2 + 1))
                rdst = rank_col if hf2 == 0 else rank2_col
                for b in range(NBLK):
                    rsc = scr.tile(
                        [128, 512], BF, tag="scrA" if hf2 == 0 else "scrB",
                        name=f"rsc{hf2}{b}",
                    )
                    nc.vector.tensor_scalar(
                        out=rsc[:],
                        in0=score2b[:, hsl2],
                        scalar1=score2_col[:, b : b + 1],
                        scalar2=None,
                        op0=A.is_gt,
                        op1=A.add,
                        accum_out=rdst[:, b : b + 1],
                    )
            nc.vector.tensor_add(
                out=rank_col[:], in0=rank_col[:], in1=rank2_col[:]
            )
            # cm = rank < 255.5 (top-256 by score), in column AND row form
            cm_col = mid.tile([128, NBLK], FP, tag="cm_col")
            nc.vector.tensor_scalar(
                out=cm_col[:], in0=rank_col[:], scalar1=float(256) - 0.5,
                scalar2=None, op0=A.is_lt,
            )
            # cmb values are exactly {0,1}, so the cheap f32r PE replicate is
            # exact (blockbcast is only needed for arbitrary fp32 rows).
            # Cluster ids are the raw center column index j* (0..1023), so no
            # prefix-sum renumbering (crank/pack) is needed at all: the
            # argmax column index IS the id, split as 32a+b with a,b in
            # 0..31 -> 64 one-hot mask rows + 64 head dims = 128 rows.
            # cmb per block via stage+transpose (no [1,N] row trip, no PE
            # replicate): ACT stage while DVE evacuates, ~3 blocks in flight
            cmb = mid.tile([128, N], FP, tag="densb", name="cmb")
            for b in range(NBLK):
                stg3 = scr.tile([128, 128], FP, tag="scrT", name=f"cmstg{b}")
                nc.scalar.activation(
                    out=stg3[:], in_=ident[:], func=AF.Identity,
                    bias=cm_col[:, b : b + 1], scale=0.0,
                )
                ptc = psS.tile([128, 128], FP, tag="ps_small", name=f"cmpt{b}")
                nc.tensor.transpose(out=ptc[:], in_=stg3[:], identity=ident[:])
                nc.vector.tensor_scalar_add(
                    out=cmb[:, 128 * b : 128 * (b + 1)], in0=ptc[:], scalar1=0.0
                )

            # ------- assignment: v_i = argmax_{centers j} z_ij (raw col) ----
            # Pool masks non-centers to 0 (centers keep z >= ~4 > 0), DVE
            # max + max_index return the winning column index directly
            # (uint32, lowest-index tie-break like the fp32 argmin).
            U32 = mybir.dt.uint32
            vmax8 = mid.tile([128, NBLK, 8], FP, tag="rmax_col", name="vmax8")
            vidx8 = mid.tile([128, NBLK, 8], U32, tag="v_col", name="vidx8")
            for b in range(NBLK):
                # alternate scratch tags -> 4 m2 slots, so Pool streams the
                # masks back-to-back instead of waiting on DVE's max_index
                m2 = scr.tile(
                    [128, N], FP, tag="scrA" if b % 2 == 0 else "scrB",
                    name=f"m2{b}",
                )
                nc.gpsimd.tensor_tensor(
                    out=m2[:], in0=zt[:, b, :], in1=cmb[:], op=A.mult
                )
                nc.vector.max(out=vmax8[:, b, :], in_=m2[:])
                nc.vector.max_index(
                    out=vidx8[:, b, :], in_max=vmax8[:, b, :], in_values=m2[:]
                )
            vf_col = mid.tile([128, NBLK], FP, tag="vf_col")
            nc.vector.tensor_copy(out=vf_col[:], in_=vidx8[:, :, 0])
            # ------- W transposes + q/k/v/colsum fill the c-stage PE idle ----
            wT = {}
            for nm in ("q", "k", "v", "p"):
                wt = consts.tile([128, 2, C], FR, tag=f"wT{nm}", name=f"wT{nm}")
                for a in range(2):
                    for b2 in range(2):
                        pt = psS.tile([128, 128], FP, tag="ps_small")
                        nc.tensor.transpose(
                            out=pt[:],
                            in_=wraw[nm][:, a, 128 * b2 : 128 * (b2 + 1)],
                            identity=ident[:],
                        )
                        nc.scalar.mul(
                            out=wt[:, b2, 128 * a : 128 * (a + 1)],
                            in_=pt[:],
                            mul=0.125 if nm == "q" else 1.0,
                        )
                wT[nm] = wt

            # rows 0:32 a-digit one-hot, 32:64 b-digit one-hot, 64:128 head
            # dims; the -256 constant is folded into the exp evac bias
            qTm = [consts.tile([128, N], FR, tag=f"qTm{h}", name=f"qTm{h}") for h in range(H)]
            kTm = [consts.tile([128, N], FR, tag=f"kTm{h}", name=f"kTm{h}") for h in range(H)]
            for dsts, wtile in ((qTm, wT["q"]), (kTm, wT["k"])):
                for m in range(2):
                    pq = psA.tile([128, N], FP, tag="ps_big")
                    for n_ in range(2):
                        sl = slice(512 * n_, 512 * (n_ + 1))
                        for k in range(2):
                            nc.tensor.matmul(
                                pq[:, sl],
                                wtile[:, k, 128 * m : 128 * (m + 1)],
                                xTr(k, sl),
                                start=(k == 0),
                                stop=(k == 1),
                            )
                    # evac into the dead xT slot (xsum hoisted earlier), so
                    # the scr pool stays free for the assignment pipeline
                    tqk = consts.tile([128, N], FR, tag="xT", name=f"tqk{m}")
                    nc.scalar.copy(out=tqk[:], in_=pq[:])
                    nc.sync.dma_start(out=dsts[2 * m][64:128, :], in_=tqk[0:64, :])
                    nc.sync.dma_start(
                        out=dsts[2 * m + 1][64:128, :], in_=tqk[64:128, :]
                    )

            va = consts.tile([128, NBLK, H, 65], FR, tag="va")
            for jb in range(NBLK):
                pv = psS.tile([128, C], FP, tag="ps_small")
                for k in range(2):
                    nc.tensor.matmul(
                        pv[:],
                        xTr(k, slice(128 * jb, 128 * (jb + 1))),
                        wT["v"][:, k, :],
                        start=(k == 0),
                        stop=(k == 1),
                    )
                nc.scalar.copy(
                    out=va[:, jb, :, 0:64],
                    in_=pv[:].rearrange("p (h d) -> p h d", h=H),
                )
                nc.vector.memset(va[:, jb, :, 64:65].bitcast(FP), 1.0)

            wpproj = consts.tile([64, H, C], FR, tag="wpproj")
            for h in range(H):
                nc.sync.dma_start(
                    out=wpproj[:, h, :],
                    in_=wT["p"][64 * (h % 2) : 64 * (h % 2) + 64, h // 2, :],
                )

            cs_sb = consts.tile([64, H], FP, tag="cs_sb")
            for m in range(2):
                pc = psS.tile([128, C], FP, tag="ps_small")
                for k in range(2):
                    nc.tensor.matmul(
                        pc[:, 0:1],
                        wT["v"][:, k, 128 * m : 128 * (m + 1)].bitcast(FP),
                        xsum[:, k : k + 1],
                        start=(k == 0),
                        stop=(k == 1),
                    )
                tpc = scr.tile([128, 1], FP, tag="scrT", name="tpc")
                nc.scalar.copy(out=tpc[:], in_=pc[:, 0:1])
                for hh in range(2):
                    nc.sync.dma_start(
                        out=cs_sb[:, 2 * m + hh : 2 * m + hh + 1],
                        in_=tpc[64 * hh : 64 * hh + 64, :],
                    )
            nc.vector.tensor_scalar_mul(out=cs_sb[:], in0=cs_sb[:], scalar1=EPS / N)

            # digits in COLUMN space: a' = round((v+16.5)/32) = floor(v/32)+1
            # via the round-to-nearest f32->u32 convert (b-15.5 stays within
            # +-0.48 of 0), b' = v - 32a' = b - 32; the one-hot iota values
            # absorb both offsets (a-rows compare 1..32, b-rows -32..-1).
            abf = mid.tile([128, 2, NBLK], FP, tag="abf")
            nc.vector.tensor_scalar(
                out=abf[:, 0, :], in0=vf_col[:], scalar1=16.5,
                scalar2=1.0 / 32.0, op0=A.add, op1=A.mult,
            )
            au_col = mid.tile([128, NBLK], U32, tag="au_col")
            nc.vector.tensor_copy(out=au_col[:], in_=abf[:, 0, :])
            nc.vector.tensor_copy(out=abf[:, 0, :], in_=au_col[:])
            nc.vector.scalar_tensor_tensor(
                out=abf[:, 1, :], in0=abf[:, 0, :], scalar=-32.0,
                in1=vf_col[:], op0=A.mult, op1=A.add,
            )
            # one transpose carries both digit rows; queue-parallel DMAs
            ptab = psS.tile([128, C], FP, tag="ps_small", name="ptab")
            nc.tensor.transpose(
                out=ptab[0:16, 0:128], in_=abf[:].rearrange("p a b -> p (a b)"),
                identity=ident[:],
            )
            tab = scr.tile([32, 128], FR, tag="scrT", name="tab")
            nc.scalar.copy(out=tab[0:16, :], in_=ptab[0:16, 0:128])
            # digits -> [64,N] via one selector matmul per block straight
            # from tab (no [1,N] row round-trip through DMA): out block b
            # rows 0:32 pick tab row b (a'), rows 32:64 pick row 8+b (b')
            pC = psA.tile([128, N], FP, tag="ps_big", name="pC")
            for b in range(NBLK):
                nc.tensor.matmul(
                    pC[0:64, 128 * b : 128 * (b + 1)],
                    selblk[:, 64 * b : 64 * (b + 1)],
                    tab[0:16, :],
                    start=True, stop=True,
                )
            # head-0 mask rows read the PSUM directly (scores start at once);
            # heads 1..3 copied in the background by DMA
            nc.vector.tensor_scalar(
                out=qTm[0][0:64, :], in0=pC[0:64, :],
                scalar1=iota64[:, 0:1], scalar2=None, op0=A.is_equal,
            )
            nc.vector.tensor_scalar(
                out=kTm[0][0:64, :], in0=pC[0:64, :],
                scalar1=iota64[:, 0:1], scalar2=BIGM, op0=A.is_equal, op1=A.mult,
            )
            for h in range(1, H):
                nc.sync.dma_start(out=qTm[h][0:64, :], in_=qTm[0][0:64, :])
                nc.sync.dma_start(out=kTm[h][0:64, :], in_=kTm[0][0:64, :])

            # ================= attention =================
            PT = big.tile([128, NBLK, N], FR, tag="zbig")  # reuses z slot
            outTs = [mid.tile([65, N], FP, tag=f"outT{h % 2}", name=f"outT{h}")
                     for h in range(H)]
            numT = [mid.tile([64, N], FR, tag=f"numT{h}", name=f"numT{h}") for h in range(H)]
            recips = [mid.tile([1, N], FR, tag=f"recip{h % 2}", name=f"recip{h}")
                      for h in range(H)]

            def head_tail(h):
                # (outT + eps_colsum) / den, in column halves; psS psum for
                # the broadcast (psPV would collide with the next head's PV
                # accumulator, bufs=1); numT reads the broadcast from PSUM
                # directly, no SBUF evac.
                for hf in range(2):
                    sl = slice(512 * hf, 512 * (hf + 1))
                    pbh = psS.tile(
                        [64, 512], FP, tag="ps_small", name=f"ht{h}{hf}"
                    )
                    nc.tensor.matmul(
                        pbh[:], ones1r[:, 0:64], recips[h][0:1, sl],
                        start=True, stop=True,
                    )
                    nc.vector.scalar_tensor_tensor(
                        out=numT[h][:, sl],
                        in0=outTs[h][0:64, sl],
                        scalar=cs_sb[:, h : h + 1],
                        in1=pbh[:],
                        op0=A.add,
                        op1=A.mult,
                    )

            for h in range(H):
                for jb in range(NBLK):
                    pst = psA.tile([128, N], FP, tag="ps_big")
                    for n_ in range(2):
                        sl = slice(512 * n_, 512 * (n_ + 1))
                        nc.tensor.matmul(
                            pst[:, sl],
                            kTm[h][:, 128 * jb : 128 * (jb + 1)],
                            qTm[h][:, sl],
                            start=True,
                            stop=True,
                        )
                    nc.scalar.activation(
                        out=PT[:, jb, :], in_=pst[:], func=AF.Exp,
                        bias=negbig[:, 0:1], scale=1.0,
                    )
                if h == H - 1:
                    # h2's tail deps are ready mid-score-phase; emit before
                    # the PV loop so its PE/DVE ops clear before ppv lands
                    head_tail(h - 1)
                ppv = psPV.tile([65, N], FP, tag="ps_pv")
                for jb in range(NBLK):
                    for n_ in range(2):
                        sl = slice(512 * n_, 512 * (n_ + 1))
                        nc.tensor.matmul(
                            ppv[:, sl],
                            va[:, jb, h, :],
                            PT[:, jb, sl],
                            start=(jb == 0),
                            stop=(jb == NBLK - 1),
                        )
                if h == 0:
                    mkrow("den")
                if h < H - 1:
                    nc.vector.tensor_scalar_add(
                        out=outTs[h][:], in0=ppv[:], scalar1=0.0
                    )
                    nc.vector.tensor_scalar(
                        out=row("den"), in0=outTs[h][64:65, :], scalar1=EPS,
                        scalar2=None, op0=A.add,
                    )
                    with nc.allow_low_precision(reason="f32r recip -> f32r bc"):
                        nc.vector.reciprocal(out=recips[h][0:1, :], in_=row("den"))
                    if h > 0:
                        head_tail(h - 1)
                else:
                    # last head: its tail is fully exposed, so pipeline the
                    # chain in column halves; den/recip read the PV PSUM row
                    # directly while the outT evac runs on now-idle ACT
                    for hf in range(2):
                        sl = slice(512 * hf, 512 * (hf + 1))
                        nc.scalar.copy(
                            out=outTs[h][:, sl], in_=ppv[:, sl]
                        )
                        nc.vector.tensor_scalar(
                            out=p0rows["den"][0:1, sl],
                            in0=ppv[64:65, sl], scalar1=EPS,
                            scalar2=None, op0=A.add,
                        )
                        with nc.allow_low_precision(reason="f32r recip bc"):
                            nc.vector.reciprocal(
                                out=recips[h][0:1, sl],
                                in_=p0rows["den"][0:1, sl],
                            )
                        pbh = psS.tile([64, 512], FP, tag="ps_small",
                                       name=f"pbh{hf}")
                        nc.tensor.matmul(
                            pbh[:], ones1r[:, 0:64], recips[h][0:1, sl],
                            start=True, stop=True,
                        )
                        nc.vector.scalar_tensor_tensor(
                            out=numT[h][:, sl],
                            in0=outTs[h][0:64, sl],
                            scalar=cs_sb[:, h : h + 1],
                            in1=pbh[:],
                            op0=A.add,
                            op1=A.mult,
                        )

            # ---------------- output projection ----------------
            # yout holds all 8 blocks so no evac ever waits on a DMA
            # completing (the old 2-slot scratch serialized the tail).
            out_r = out_ext.rearrange("(b p) c -> p b c", p=128)
            # two 4-block staging tiles on dead tags: no evac ever waits on
            # an output DMA completing
            yout_lo = mid.tile([128, 4, C], FP, tag="densb", name="yout_lo3")
            yout_hi = mid.tile([128, 4, C], FP, tag="outT0", name="yout_hi")
            for ib in range(NBLK):
                py = psS.tile([128, C], FP, tag="ps_small")
                for h in range(H):
                    nc.tensor.matmul(
                        py[:],
                        numT[h][:, 128 * ib : 128 * (ib + 1)],
                        wpproj[:, h, :],
                        start=(h == 0),
                        stop=False,
                    )
                nc.tensor.matmul(
                    py[:], ones1r[:], bp_fr[:], start=False, stop=True
                )
                yo = yout_lo if ib < 4 else yout_hi
                nc.scalar.copy(out=yo[:, ib % 4, :], in_=py[:])
                if ib % 2 == 1:
                    s4 = (ib - 1) % 4
                    nc.sync.dma_start(
                        out=out_r[:, ib - 1 : ib + 1, :],
                        in_=yo[:, s4 : s4 + 2, :],
                    )

    return nc


_CACHE = {}


def _get_nc():
    if "nc" not in _CACHE:
        _CACHE["nc"] = build_nc()
        _CACHE["noise"] = _noise_cols()
        _CACHE["ident"] = np.eye(128, dtype=np.float32)
        _CACHE["iota64"] = np.concatenate(
            [np.arange(32) + 1, np.arange(32) - 32]
        ).reshape(64, 1).astype(np.float32)
        selblk = np.zeros((16, 512), dtype=np.float32)
        for _b in range(8):
            selblk[_b, 64 * _b : 64 * _b + 32] = 1.0
            selblk[8 + _b, 64 * _b + 32 : 64 * _b + 64] = 1.0
        _CACHE["selblk"] = selblk
    return _CACHE["nc"]


def kernel(x_token, Wq, Wk, Wv, Wp, bp, _trace=False, _trace_kwargs=None):
    from concourse.bass_utils import run_bass_kernel_spmd

    nc = _get_nc()
    noise = _CACHE["noise"]
    x_token = np.ascontiguousarray(np.asarray(x_token, dtype=np.float32))
    weights = {
        "Wq": np.ascontiguousarray(np.asarray(Wq, dtype=np.float32)),
        "Wk": np.ascontiguousarray(np.asarray(Wk, dtype=np.float32)),
        "Wv": np.ascontiguousarray(np.asarray(Wv, dtype=np.float32)),
        "Wp": np.ascontiguousarray(np.asarray(Wp, dtype=np.float32)),
        "bp": np.ascontiguousarray(np.asarray(bp, dtype=np.float32)),
    }
    consts = {
        "ident": _CACHE["ident"],
        "iota64": _CACHE["iota64"],
        "selblk": _CACHE["selblk"],
    }
    in_maps = []
    for b in range(B):
        in_maps.append(dict(weights, x=x_token[b], noise=noise[b], **consts))
    kw = {}
    if _trace:
        kw = dict(trace=True, trace_kwargs=_trace_kwargs or {})
    res = run_bass_kernel_spmd(nc, in_maps, list(range(B)), **kw)
    out = np.stack([res.results[b]["out"] for b in range(B)], axis=0)
    if _trace:
        return out, res
    return out

